# revision 18
# baseline (speedup 1.0000x reference)
"""Trainium2 Bass kernel: NeuralNearestNeighbors continuous-KNN weight volumes.

Reference computation (per row of D.reshape(b*m, o), K=8 rounds):
    logits = D / exp(log_temp)
    for k in range(K):
        w_k = log_softmax(logits);  out_k = exp(w_k)
        logits = logits + log1mexp(w_k)          # log(1 - p_k)
    W = stack(out_k, axis=-1)                     # (b, m, o, K)

Exp-space identity: with p_k = softmax(logits_k) and F_k = p_k, keep a
sign-flipped unnormalized state with per-row scalar g, F = state * g:
    G_0 = exp(D/T)          a_0 = sum(G_0)          g_0 = 1/a_0   (>0)
    G_{k+1} = (F_k - 1)*F_k a_{k+1} = t_k - 1 < 0   g = 1/a       (<0)
Rounds alternate two equivalent state forms so pass2 splits across engines:
  G-rounds (DVE stt):   G' = (F-1)*F, bf16, free accum -> a
  H-rounds (ACT):       H = Square(F - 0.5) = G' + 1/4, f32, accum -> s;
                        a = s - O/4 = s - 128; F = (H - 0.25)*g
pass1 is always a DVE tensor_scalar (1-scalar for G, 2-scalar for H),
written k-major into a [P, K, O] bf16 out tile.  Everything contiguous.
Output goes to HBM as [rows, K, O]; the host does the [K, O] -> [O, K]
interleave + f32 upcast during unshard.  Emission is interleaved over
GROUP row-tiles, reciprocals/fixups batched [P,2], output-DMA triggers on
the idle SP sequencer, and the last group's stores are split in half so
the DMA tail shrinks.

Sharding: purely rowwise data-parallel over b*m = 16384 rows; 2048 rows per
core across 8 cores; log_temp replicated.
"""

import numpy as np

B, M, O = 16, 1024, 512
K = 8
N_CORES = 8
ROWS = B * M                     # 16384
RPC = ROWS // N_CORES            # 2048 rows per core
P = 128
TILES = RPC // P                 # 16 row-tiles per core
GROUPS = ((0, 4), (4, 4), (8, 8))  # (base, size): small first group ramps fast
IN_DMA_GROUP = 4                 # row-tiles per input DMA
H_ROUNDS_A = (1, 3, 5, 7)        # tile half A: pass2 on ACT at odd rounds
H_ROUNDS_B = (2, 4, 6, 7)        # tile half B: pass2 on ACT at even rounds


def _is_h(i, n, k):
    """Whether tile i (of an n-tile group) uses the ACT/Square form at k."""
    return k in (H_ROUNDS_A if i < n // 2 else H_ROUNDS_B)

_cached = None


def _build(reps=1, variant="mixed"):
    """Build and compile the Bass module (one SPMD program for all cores)."""
    from contextlib import ExitStack

    import concourse.bacc as bacc
    import concourse.tile as tile
    from concourse import mybir

    f32 = mybir.dt.float32
    bf16 = mybir.dt.bfloat16
    Alu = mybir.AluOpType
    Act = mybir.ActivationFunctionType

    nc = bacc.Bacc(
        "TRN2",
        target_bir_lowering=False,
        debug=False,
        enable_asserts=False,
        num_devices=N_CORES,
    )
    d = nc.dram_tensor("d", [RPC, O], f32, kind="ExternalInput").ap()
    lt = nc.dram_tensor("log_temp", [1, 1], f32, kind="ExternalInput").ap()
    # HBM layout is [row, k, o] (k-major); host swaps the last two axes.
    w = nc.dram_tensor("w", [RPC, K * O], bf16, kind="ExternalOutput").ap()

    with tile.TileContext(nc) as tc, ExitStack() as ctx:
        singles = ctx.enter_context(tc.tile_pool(name="singles", bufs=1))
        slab_pool = ctx.enter_context(tc.tile_pool(name="slab", bufs=1))
        out_pool = ctx.enter_context(tc.tile_pool(name="out", bufs=TILES))
        small = ctx.enter_context(tc.tile_pool(name="small", bufs=24))

        # log_temp -> 1/T = exp(-log_temp), replicated to all 128 partitions.
        lt_sb = singles.tile([P, 1], f32)
        nc.sync.dma_start(out=lt_sb[:, :], in_=lt.to_broadcast((P, 1)))
        invt = singles.tile([P, 1], f32)
        nc.scalar.activation(invt[:, :], lt_sb[:, :], Act.Exp, scale=-1.0)
        # touch DVE immediately so its uop TENSOR_LOAD happens in the
        # launch shadow instead of before the first real vector op.
        dve_warm = singles.tile([P, 1], f32)
        nc.vector.tensor_scalar_add(dve_warm[:, :], lt_sb[:, :], 0.0)
        # [P,1] constant for the Square bias (only 0.0/1.0 are pre-registered).
        neghalf = singles.tile([P, 1], f32)
        nc.gpsimd.memset(neghalf[:, :], -0.5)

        din = d.rearrange("(t p) o -> p t o", p=P)

        def body():
            # Input lands in the f32 H slab (exp reads it before any H
            # write of the same tile), bf16 G slab holds G-state.
            gslab = slab_pool.tile([P, TILES, O], bf16)
            hslab = slab_pool.tile([P, TILES, O], f32)
            in_chunks = [(0, 2), (2, 2)] + [
                (s, IN_DMA_GROUP) for s in range(4, TILES, IN_DMA_GROUP)
            ]
            for gstart, glen in in_chunks:
                # SWDGE path: keeps the HWDGE ring free for output writes.
                nc.gpsimd.dma_start(
                    out=hslab[:, gstart : gstart + glen, :],
                    in_=din[:, gstart : gstart + glen, :],
                )

            def pass1(t, ct_t, gam_i, k, i, n):
                # F_k = G_k * g_k  (G-state)  or  (H_k - 1/4) * g_k  (H-state)
                f_k = ct_t[:, k, :]
                if k > 0 and _is_h(i, n, k):
                    nc.vector.tensor_scalar(
                        f_k, hslab[:, t, :], 0.25, gam_i,
                        Alu.subtract, Alu.mult,
                    )
                elif k == 0 and i < n // 2:
                    # half of round 0 rides the ACT engine for balance
                    nc.scalar.mul(f_k, gslab[:, t, :], gam_i)
                else:
                    nc.vector.tensor_scalar(
                        f_k, gslab[:, t, :], gam_i, None, Alu.mult
                    )

            def emit_exps(grp, acc4):
                # ACT side only: exp + row-sum accum for each tile
                for i, t in enumerate(grp):
                    nc.scalar.activation(
                        gslab[:, t, :], hslab[:, t, :], Act.Exp,
                        scale=invt[:, :], accum_out=acc4[:, i : i + 1],
                    )

            def emit_exp_recips(acc4, gam4, n):
                for i in range(1, n, 2):
                    nc.vector.reciprocal(
                        gam4[:, i - 1 : i + 1], acc4[:, i - 1 : i + 1]
                    )

            # group 0 prologue
            n0 = GROUPS[0][1]
            acc4 = small.tile([P, n0], f32, name="acc_p0", tag="sm")
            gam4 = small.tile([P, n0], f32, name="gam_p0", tag="sm")
            emit_exps(list(range(n0)), acc4)
            emit_exp_recips(acc4, gam4, n0)
            pending = (acc4, gam4)

            for gi, (base, n) in enumerate(GROUPS):
                grp = list(range(base, base + n))
                nxt = list(range(*GROUPS[gi + 1])) if gi + 1 < len(GROUPS) \
                    else None
                if nxt is not None:
                    nxt = list(range(GROUPS[gi + 1][0],
                                     GROUPS[gi + 1][0] + GROUPS[gi + 1][1]))
                last = nxt is None
                ct = {t: out_pool.tile([P, K, O], bf16, name=f"c{t}", tag="c")
                      for t in grp}
                acc4, gam4 = pending
                for i, t in enumerate(grp):
                    pass1(t, ct[t], gam4[:, i : i + 1], 0, i, n)
                for k in range(1, K):
                    acc4 = small.tile([P, n], f32, name=f"acc_{k}", tag="sm")
                    gam4 = small.tile([P, n], f32, name=f"gam_{k}", tag="sm")
                    dk4 = small.tile([P, n], f32, name=f"dk_{k}", tag="sm")
                    # G-half first so DVE streams stt while ACT runs squares
                    gh = [(i, t) for i, t in enumerate(grp)
                          if not _is_h(i, n, k)]
                    hh = [(i, t) for i, t in enumerate(grp) if _is_h(i, n, k)]
                    for i, t in hh:
                        nc.scalar.activation(
                            hslab[:, t, :], ct[t][:, k - 1, :], Act.Square,
                            bias=neghalf[:, :],
                            accum_out=acc4[:, i : i + 1],
                        )
                    for j, (i, t) in enumerate(gh):
                        nc.vector.scalar_tensor_tensor(
                            out=gslab[:, t, :],
                            in0=ct[t][:, k - 1, :],
                            scalar=1.0,
                            in1=ct[t][:, k - 1, :],
                            op0=Alu.subtract,
                            op1=Alu.mult,
                            accum_out=acc4[:, i : i + 1],
                        )
                        if j % 2 == 1:
                            i0 = gh[j - 1][0]
                            assert i0 + 1 == i
                            nc.vector.reciprocal(
                                gam4[:, i0 : i + 1], acc4[:, i0 : i + 1]
                            )
                    for j, (i, t) in enumerate(hh):
                        if j % 2 == 1:
                            i0 = hh[j - 1][0]
                            assert i0 + 1 == i
                            nc.vector.tensor_scalar_add(
                                dk4[:, i0 : i + 1], acc4[:, i0 : i + 1], -128.0
                            )
                            nc.vector.reciprocal(
                                gam4[:, i0 : i + 1], dk4[:, i0 : i + 1]
                            )
                    for i, t in gh:
                        pass1(t, ct[t], gam4[:, i : i + 1], k, i, n)
                    for i, t in hh:
                        pass1(t, ct[t], gam4[:, i : i + 1], k, i, n)
                    if k == 5 and nxt is not None:
                        # hoist next group's exps into ACT's slack
                        nn = len(nxt)
                        nacc = small.tile([P, nn], f32, name="acc_nx", tag="sm")
                        ngam = small.tile([P, nn], f32, name="gam_nx", tag="sm")
                        emit_exps(nxt, nacc)
                        pending = (nacc, ngam)
                    if k == 6 and nxt is not None:
                        emit_exp_recips(pending[0], pending[1], len(nxt))
                    if last and k == 3:
                        for t in grp:
                            nc.sync.dma_start(
                                out=w[t * P : (t + 1) * P, : 4 * O],
                                in_=ct[t][:, :4, :],
                            )
                    if last and k == 5:
                        for t in grp:
                            nc.sync.dma_start(
                                out=w[t * P : (t + 1) * P, 4 * O : 6 * O],
                                in_=ct[t][:, 4:6, :],
                            )
                if last:
                    for t in grp:
                        nc.sync.dma_start(
                            out=w[t * P : (t + 1) * P, 6 * O :],
                            in_=ct[t][:, 6:, :],
                        )
                else:
                    for t in grp:
                        nc.sync.dma_start(
                            out=w[t * P : (t + 1) * P, :], in_=ct[t][:, :, :]
                        )

        if reps > 1:
            with tc.For_i(
                0, reps, 1,
                hint_engines=(
                    mybir.EngineType.DVE,
                    mybir.EngineType.Activation,
                    mybir.EngineType.SP,
                ),
            ):
                body()
        else:
            body()

    nc.compile()
    return nc


VARIANT = "mixed"


def _get_nc():
    global _cached
    if _cached is None:
        _cached = _build(variant=VARIANT)
    return _cached


def _make_in_maps(D, log_temp):
    Dr = np.ascontiguousarray(np.asarray(D, dtype=np.float32).reshape(ROWS, O))
    lt = np.asarray(log_temp, dtype=np.float32).reshape(1, 1)
    return [
        {"d": Dr[c * RPC : (c + 1) * RPC], "log_temp": lt}
        for c in range(N_CORES)
    ]


def _gather(results):
    # per-core HBM layout is [row, k, o]; swap to [row, o, k] + upcast here.
    parts = [
        np.asarray(results[c]["w"]).reshape(RPC, K, O) for c in range(N_CORES)
    ]
    full = np.stack(parts, axis=0).astype(np.float32)   # (C, RPC, K, O)
    return np.ascontiguousarray(full.transpose(0, 1, 3, 2)).reshape(B, M, O, K)


def run_spmd(D, log_temp, trace=False, **kwargs):
    """Run on all 8 cores; returns (W, BassKernelResults)."""
    from concourse.bass_utils import run_bass_kernel_spmd

    nc = _get_nc()
    res = run_bass_kernel_spmd(
        nc, _make_in_maps(D, log_temp), list(range(N_CORES)), trace=trace, **kwargs
    )
    return _gather(res.results), res


def kernel(D, log_temp):
    W, _ = run_spmd(D, log_temp)
    return W
